# revision 12
# baseline (speedup 1.0000x reference)
"""Distributed exact inner-product top-k (brute-force kNN) on 8 TRN2 NeuronCores.

Sharding: codebook W is split row-wise into 8 shards of 25000 (one per core);
x is replicated.  Host pre-transposes both to bf16: the PE streams 1 output
column/cycle regardless of dtype at contraction 128 (fp8 DoubleRow only
pays off at contraction 256 - measured), so bf16 is free accuracy.

Device kernel (SPMD, identical graph per core, no collectives):
  - per 1024-col region (2 PSUM banks, 4 in flight so both drain engines
    always have a ready region): 2x bf16 matmuls [128 contraction,
    512 cols] into PSUM (f32)
  - each region is drained by one of the only two engines that can read
    PSUM, statically balanced by their clocks and overheads (measured
    1090ns/region DVE vs 1038ns Act):
      D: DVE windowed tensor_reduce(max) w=4 -> bf16 window maxima
         (even regions, 12 per group)
      A: Act copy PSUM -> fp8e4 raw scores, window-1 (odd regions + the
         424-col tail)
  - per-row outputs: 3072 bf16 w4-maxima + 12712 fp8 raw scores, DMA'd in
    multi-region chunks alternating between the SP and gpsimd DMA queues

Host merge (the all-gather + final top-k of the distributed ANN pattern):
  - per row, select every window whose (value + its route's EPS) clears
    (128th-largest window value - EPSMAX - slack); gather member columns
  - exact f64 re-rank of the candidates; final top-128 ordered like
    jax.lax.top_k (value desc, index asc)
  - exactness guard: containment holds if |device value - exact window max|
    <= EPS_route for every window that can matter; all such windows are
    selected, EPS is validated on them per-run, and violating rows
    (expected none) are recomputed exactly.  bf16 inputs keep the gap tiny
    (bf16-out windows ~0.25, fp8e4-out windows ~2.2 at |s|~45), so margins
    and candidate counts stay small and the host merge is cheap.
"""

import numpy as np

B = 1024
D = 128
VOCAB = 200000
NCORES = 8
VSHARD = VOCAB // NCORES  # 25000
REG = 1024  # 2 PSUM banks of f32
NREG = 24  # full regions per shard
TAIL = VSHARD - NREG * REG  # 424 -> Act
TOPK = 128
NGRP = B // 128

# Region schedule per group: even regions -> DVE w4 reduce ("D"), odd
# regions and the 424-col tail -> Act fp8 copy ("A"); region 22 is split
# half/half ("S").  This lands on the measured drain balance
# (DVE ~1153ns/region at 0.96GHz vs Act ~1018ns at 1.2GHz).
SPLIT_R = 22
SPLIT_D = 920
ROUTES = ["D" if r % 2 == 0 else "A" for r in range(NREG)] + ["A"]
ROUTES[SPLIT_R] = "S"
W4_SEGS = [(r * REG, REG) for r in range(0, NREG, 2) if r != SPLIT_R] + [
    (SPLIT_R * REG, SPLIT_D)
]
W1_SEGS = (
    [(r * REG, REG) for r in range(1, NREG, 2)]
    + [(SPLIT_R * REG + SPLIT_D, REG - SPLIT_D)]
    + [(NREG * REG, TAIL)]
)
W4_SEGS.sort()
W1_SEGS.sort()
NW4 = sum(n // 4 for _, n in W4_SEGS)  # 3072
NW1 = sum(n for _, n in W1_SEGS)  # 12712
W4_OFF = np.concatenate([[0], np.cumsum([n // 4 for _, n in W4_SEGS])])
W1_OFF = np.concatenate([[0], np.cumsum([n for _, n in W1_SEGS])])

# |device window value - exact window max| bounds, validated at runtime:
# bf16 input quantization noise on x and W (~0.1-0.2) plus output
# quantization (bf16 ~0.2 for route D, fp8e4 ~2.2 for route A at |s|~45).
EPS4 = 0.45
EPS1 = 2.5
EPSMAX = EPS1
SLACK = 0.3

LAST_RESULTS = None  # BassKernelResults of the most recent run (for profiling)
_CACHED_NC = None


def build_kernel():
    import concourse.bass as bass  # noqa: F401
    import concourse.tile as tile
    from concourse import bacc, mybir

    F32 = mybir.dt.float32
    BF16 = mybir.dt.bfloat16
    FP8 = mybir.dt.float8e4
    AX = mybir.AxisListType.X
    MAX = mybir.AluOpType.max
    COPY = mybir.ActivationFunctionType.Copy

    nc = bacc.Bacc("TRN2", target_bir_lowering=False, debug=False)
    wt_d = nc.dram_tensor("wt", [D, VSHARD], BF16, kind="ExternalInput")
    xt_d = nc.dram_tensor("xt", [D, B], BF16, kind="ExternalInput")
    out4_d = nc.dram_tensor("out_w4", [B, NW4], BF16, kind="ExternalOutput")
    out1_d = nc.dram_tensor("out_w1", [B, NW1], FP8, kind="ExternalOutput")

    with tile.TileContext(nc) as tc:
        with (
            tc.tile_pool(name="wt", bufs=1) as wt_pool,
            tc.tile_pool(name="xt", bufs=1) as xt_pool,
            tc.tile_pool(name="psum", bufs=4, space="PSUM") as psum_pool,
            tc.tile_pool(name="out4", bufs=4) as out4_pool,
            tc.tile_pool(name="out1", bufs=4) as out1_pool,
        ):
            wt_sb = wt_pool.tile([D, VSHARD], BF16)
            xt_sb = xt_pool.tile([D, B], BF16)
            # xt on SP, first W slab concurrently on the gpsimd queue.
            nc.sync.dma_start(xt_sb[:], xt_d[:])
            slabs = [512] * 8 + [1024] * 20 + [424]
            assert sum(slabs) == VSHARD
            lo = 0
            for s, w in enumerate(slabs):
                eng = nc.gpsimd if s % 2 == 0 else nc.sync
                eng.dma_start(wt_sb[:, lo:lo + w], wt_d[:, lo:lo + w])
                lo += w

            # Per-region output window offsets (segments are column-sorted
            # and regions are processed in column order, so offsets align).
            o4_of = {}
            o1_of = {}
            for si, (slo, n) in enumerate(W4_SEGS):
                o4_of[slo] = int(W4_OFF[si])
            for si, (slo, n) in enumerate(W1_SEGS):
                o1_of[slo] = int(W1_OFF[si])

            # DMA-out cuts: (after_region, stream, win_lo, win_hi)
            def done4(r):
                return sum(
                    n // 4 for slo, n in W4_SEGS if slo < (r + 1) * REG
                )

            def done1(r):
                return sum(n for slo, n in W1_SEGS if slo < (r + 1) * REG)

            cuts = []
            prev1 = 0
            for cr in (5, 11, 17, 21, NREG):
                hi = done1(cr)
                cuts.append((cr, 1, prev1, hi))
                prev1 = hi
            prev4 = 0
            for cr in (12, 20, NREG):
                hi = done4(cr)
                cuts.append((cr, 4, prev4, hi))
                prev4 = hi

            for g in range(NGRP):
                out4_sb = out4_pool.tile([128, NW4], BF16, tag="out4")
                out1_sb = out1_pool.tile([128, NW1], FP8, tag="out1")
                xg = xt_sb[:, g * 128:(g + 1) * 128]
                for r in range(NREG + 1):
                    base = r * REG
                    w_cols = REG if r < NREG else TAIL
                    route = ROUTES[r]
                    ps = psum_pool.tile([128, REG], F32)
                    for k in range(0, w_cols, 512):
                        kw = min(512, w_cols - k)
                        nc.tensor.matmul(
                            ps[:, k:k + kw],
                            xg,
                            wt_sb[:, base + k:base + k + kw],
                            start=True, stop=True,
                        )
                    if route in ("D", "S"):
                        nd = w_cols if route == "D" else SPLIT_D
                        o4 = o4_of[base]
                        nc.vector.tensor_reduce(
                            out4_sb[:, o4:o4 + nd // 4],
                            ps[:, :nd].rearrange("p (n w) -> p n w", w=4),
                            axis=AX, op=MAX,
                        )
                    if route in ("A", "S"):
                        lo1 = 0 if route != "S" else SPLIT_D
                        o1 = o1_of[base + lo1]
                        nc.scalar.activation(
                            out1_sb[:, o1:o1 + w_cols - lo1],
                            ps[:, lo1:w_cols],
                            COPY,
                        )
                    for ci, (cr, which, wlo, whi) in enumerate(cuts):
                        if cr != r:
                            continue
                        eng = nc.sync if (g + ci) % 2 == 0 else nc.gpsimd
                        src_t = out4_sb if which == 4 else out1_sb
                        dst = out4_d if which == 4 else out1_d
                        eng.dma_start(
                            dst[g * 128:(g + 1) * 128, wlo:whi],
                            src_t[:, wlo:whi],
                        )
    nc.compile()
    return nc


def _build_maps():
    """Per-window candidate columns and EPS.

    Returns (colmap [NWIN, 4] int64 with -1 pads, eps [NWIN] f32) where
    window order is [all w4 windows, all w1 windows] per core.
    """
    nwin = NW4 + NW1
    cm = np.full((nwin, 4), -1, np.int64)
    eps = np.empty(nwin, np.float32)
    for si, (lo, n) in enumerate(W4_SEGS):
        o = int(W4_OFF[si])
        j = np.arange(n // 4)[:, None]
        cm[o:o + n // 4] = lo + 4 * j + np.arange(4)[None, :]
    eps[:NW4] = EPS4
    for si, (lo, n) in enumerate(W1_SEGS):
        o = NW4 + int(W1_OFF[si])
        cm[o:o + n, 0] = lo + np.arange(n)
    eps[NW4:] = EPS1
    return cm, eps


_COLMAP, _WEPS = _build_maps()


def _topk_rows(vals, gidx, k):
    """Per-row top-k ordered like jax.lax.top_k: value desc, index asc."""
    order = np.lexsort((gidx, -vals), axis=-1)[:, :k]
    return (
        np.take_along_axis(gidx, order, axis=1),
        np.take_along_axis(vals, order, axis=1),
    )


def kernel(x: np.ndarray, W: np.ndarray, topk) -> np.ndarray:
    global LAST_RESULTS, _CACHED_NC
    import os

    import ml_dtypes

    from concourse.bass_utils import run_bass_kernel_spmd

    assert x.shape == (B, D) and W.shape == (VOCAB, D)
    assert int(topk) == TOPK
    x = np.ascontiguousarray(np.asarray(x, dtype=np.float32))
    W = np.ascontiguousarray(np.asarray(W, dtype=np.float32))

    if _CACHED_NC is None:
        _CACHED_NC = build_kernel()
    nc = _CACHED_NC

    xt = np.ascontiguousarray(x.T).astype(ml_dtypes.bfloat16)
    in_maps = []
    for i in range(NCORES):
        wt_i = np.ascontiguousarray(
            W[i * VSHARD:(i + 1) * VSHARD].T
        ).astype(ml_dtypes.bfloat16)
        in_maps.append({"wt": wt_i, "xt": xt})

    LAST_RESULTS = run_bass_kernel_spmd(
        nc,
        in_maps,
        core_ids=list(range(NCORES)),
        trace=bool(int(os.environ.get("KERNEL_TRACE", "0"))),
    )
    results = LAST_RESULTS.results

    # [B, 8*(NW4+NW1)] device window values, f32
    nwin = NW4 + NW1
    wm = np.empty((B, NCORES * nwin), np.float32)
    for i in range(NCORES):
        wm[:, i * nwin:i * nwin + NW4] = np.asarray(
            results[i]["out_w4"]).astype(np.float32)
        wm[:, i * nwin + NW4:(i + 1) * nwin] = np.asarray(
            results[i]["out_w1"]).astype(np.float32)
    nwin_all = NCORES * nwin
    weps_all = np.tile(_WEPS, NCORES)

    # Per-row selection on adjusted values v' = v + eps_w:
    # keep windows with v' >= kth_dev - EPSMAX - SLACK.
    wma = wm + weps_all[None, :]
    kth = np.partition(wm, nwin_all - TOPK, axis=1)[:, nwin_all - TOPK]
    tau = kth - EPSMAX - SLACK
    counts = (wma >= tau[:, None]).sum(axis=1)
    K = int(min(max(int(counts.max()), TOPK + 64), 4096))
    topw = np.argpartition(-wma, K - 1, axis=1)[:, :K]  # [B, K] window ids

    core_id = topw // nwin
    wi = topw % nwin
    cols = _COLMAP[wi]  # [B, K, 4], -1 = pad
    pad = cols < 0
    cand = (np.where(pad, 0, cols) + core_id[..., None] * VSHARD).reshape(B, K * 4)

    # Exact f64 re-rank of the candidate columns (pads scored -inf).
    x64 = x.astype(np.float64)
    W64 = W.astype(np.float64)
    exact = np.empty((B, K * 4), np.float64)
    STEP = 64
    for r0 in range(0, B, STEP):
        r1 = r0 + STEP
        gW = W64[cand[r0:r1]]  # [STEP, K*4, D]
        exact[r0:r1] = np.einsum("bjd,bd->bj", gW, x64[r0:r1])
    exact[pad.reshape(B, K * 4)] = -np.inf

    # Rank on f32-rounded scores so near-ties break the same way as the
    # f32 reference (top_k on an f32 score matrix, ties by index asc).
    gidx_top, vals_top = _topk_rows(
        exact.astype(np.float32).astype(np.float64), cand, TOPK)

    # Exactness guards: EPS must hold on every selected window (any window
    # that can contain a true top-128 column is selected), and the
    # selection count must fit in K.
    dev_w = np.take_along_axis(wm, topw, axis=1)
    true_w = exact.reshape(B, K, 4).max(axis=2)
    werr = np.abs(dev_w - true_w)
    sel_eps = weps_all[topw]
    err_excess = (werr - sel_eps).max(axis=1)
    bad = (err_excess > 0) | (counts > K)
    if os.environ.get("KERNEL_DEBUG"):
        w4mask = (topw % nwin) < NW4
        e4 = werr[w4mask].max() if w4mask.any() else 0.0
        e1 = werr[~w4mask].max() if (~w4mask).any() else 0.0
        print(f"[kernel] K={K} counts max={counts.max()} "
              f"err4 max={e4:.3f} err1 max={e1:.3f} bad rows={int(bad.sum())}")
    for r in np.flatnonzero(bad):
        s = x64[r] @ W64.T
        gidx_top[r] = np.lexsort((np.arange(VOCAB), -s))[:TOPK]

    return gidx_top.astype(np.int32)


# revision 13
# speedup vs baseline: 1.0922x; 1.0922x over previous
"""Distributed exact inner-product top-k (brute-force kNN) on 8 TRN2 NeuronCores.

Sharding: codebook W is split row-wise into 8 shards of 25000 (one per core);
x is replicated.  Host pre-transposes both (fp8e4) so the contraction dim
(128) lands on SBUF partitions.

Device kernel (SPMD, identical graph per core, no collectives):
  - per 1024-wide vocab region (2 PSUM banks): 2x fp8 matmuls [128 rows, 512]
    into PSUM (f32 accumulation); 4 PSUM tiles keep the PE 4 regions ahead
  - each region's 1024 f32 scores are then drained by one of two routes so
    the work is split across the only two engines that can read PSUM:
      A: DVE windowed tensor_reduce(max) [128,256,4] -> 256 window-4 maxima
      D: Act copy PSUM->out tile bf16 (raw)          -> 1024 window-1 values
    (GPSIMD/Pool cannot read PSUM on TRN2 and cannot run TensorTensor in this
    toolchain; DMA cannot read PSUM nor max-accumulate — verified empirically)
  - the per-row stream of 16234 window maxima (bf16) is DMA'd out in chunks
    alternating between the SP and gpsimd DMA queues

Host merge (the all-gather + final top-k of the distributed ANN pattern):
  - per row, select every window whose max clears (128th-largest window max
    - MARGIN); gather those windows' member columns as candidates
  - exact f64 re-rank of the candidates; final top-128 ordered like
    jax.lax.top_k (value desc, index asc)
  - exactness guard: MARGIN >= 2*EPS guarantees containment of the true
    top-128 given |device window max - exact window max| <= EPS; EPS is
    validated per-run on every selected window (device value vs exact f64
    value), and violating rows (expected none) are recomputed exactly.
"""

import numpy as np

B = 1024
D = 128
VOCAB = 200000
NCORES = 8
VSHARD = VOCAB // NCORES  # 25000
REGION = 1024  # 2 PSUM banks of f32
NREG = 24  # full regions per shard
VPAD = VSHARD
TAIL = VSHARD - NREG * REGION  # 424
TOPK = 128

# Engine route per full region (tail is always 'A'):
#   A = DVE windowed reduce (window 4)
#   C = Act copy + one DVE bf16 fold (window 2) — unused in the final mix
#   D = Act copy straight to the out tile (window 1, raw)
ROUTES = "DA" * 11 + "DD"
assert len(ROUTES) == NREG

_WINS = {"A": REGION // 4, "C": REGION // 2, "D": REGION}


def _region_wins(r: int) -> int:
    return _WINS[ROUTES[r]] if r < NREG else TAIL // 4


# output offset of each region's windows in the out tile
WOFF = np.concatenate([[0], np.cumsum([_region_wins(r) for r in range(NREG + 1)])])
NWIN = int(WOFF[-1])

# |device window max - exact window max| bound: fp8e4 input quantization
# noise (std ~0.6, observed max ~3) + bf16 output quantization (~0.2 at
# score ~45).  Validated at runtime on every selected window.
EPS_BOUND = 3.5
MARGIN = 7.5  # >= 2*EPS_BOUND + slack

LAST_RESULTS = None  # BassKernelResults of the most recent run (for profiling)
_CACHED_NC = None


def build_kernel():
    import concourse.bass as bass  # noqa: F401
    import concourse.tile as tile
    from concourse import bacc, mybir

    F32 = mybir.dt.float32
    BF16 = mybir.dt.bfloat16
    FP8 = mybir.dt.float8e4
    AX = mybir.AxisListType.X
    MAX = mybir.AluOpType.max
    COPY = mybir.ActivationFunctionType.Copy

    nc = bacc.Bacc("TRN2", target_bir_lowering=False, debug=False)
    wt_d = nc.dram_tensor("wt", [D, VPAD], FP8, kind="ExternalInput")
    xt_d = nc.dram_tensor("xt", [D, B], FP8, kind="ExternalInput")
    out_d = nc.dram_tensor("out_win", [B, NWIN], BF16, kind="ExternalOutput")

    with tile.TileContext(nc) as tc:
        with (
            tc.tile_pool(name="wt", bufs=1) as wt_pool,
            tc.tile_pool(name="xt", bufs=1) as xt_pool,
            tc.tile_pool(name="psum", bufs=4, space="PSUM") as psum_pool,
            tc.tile_pool(name="outw", bufs=4) as out_pool,
        ):
            wt_sb = wt_pool.tile([D, VPAD], FP8)
            xt_sb = xt_pool.tile([D, B], FP8)
            # xt first: the first matmul's stationary operand should not wait
            # behind the whole 3.2MB W load.  W is split into slabs in
            # consumption order, alternating between the SP HW queue and the
            # gpsimd software-DGE queue (both engines otherwise idle, so the
            # DMA-trigger instruction cost stays off the critical engines).
            nc.sync.dma_start(xt_sb[:], xt_d[:])
            # small leading slabs so the first regions' weights land fast
            slabs = [512] * 8
            rest = VPAD - sum(slabs)
            nrest = 24
            for s in range(nrest):
                slabs.append(rest // nrest + (1 if s < rest % nrest else 0))
            lo = 0
            for s, w in enumerate(slabs):
                eng = nc.sync if s % 2 == 0 else nc.gpsimd
                eng.dma_start(wt_sb[:, lo:lo + w], wt_d[:, lo:lo + w])
                lo += w
            assert lo == VPAD

            # out DMA is chunked after these regions so the transfer of a
            # group's early windows overlaps the rest of the group's compute;
            # cuts alternate between the SP and gpsimd queues by group parity
            # so the two output streams drain in parallel
            cut_regs = [2, 5, 8, 11, 14, 17, 20, NREG]
            DMA_CUTS = {}
            prev = 0
            for cr in cut_regs:
                DMA_CUTS[cr] = (prev, int(WOFF[cr + 1]))
                prev = int(WOFF[cr + 1])

            for g in range(B // 128):
                out_sb = out_pool.tile([128, NWIN], BF16, tag="outw")
                xg = xt_sb[:, g * 128:(g + 1) * 128]
                for r in range(NREG + 1):
                    base = r * REGION
                    w_cols = REGION if r < NREG else TAIL
                    route = ROUTES[r] if r < NREG else "A"
                    wo = int(WOFF[r])
                    ps = psum_pool.tile([128, REGION], F32)
                    for k in range(0, w_cols, 512):
                        kw = min(512, w_cols - k)
                        nc.tensor.matmul(
                            ps[:, k:k + kw],
                            xg,
                            wt_sb[:, base + k:base + k + kw],
                            start=True, stop=True,
                        )
                    owin = out_sb[:, wo:wo + _region_wins(r)]
                    if route == "A":
                        nc.vector.tensor_reduce(
                            owin,
                            ps[:, :w_cols].rearrange("p (n w) -> p n w", w=4),
                            axis=AX, op=MAX,
                        )
                    else:  # "D"
                        nc.scalar.activation(owin, ps[:], COPY)
                    if r in DMA_CUTS:
                        lo, hi = DMA_CUTS[r]
                        ci = cut_regs.index(r)
                        eng = nc.sync if (g + ci) % 2 == 0 else nc.gpsimd
                        eng.dma_start(
                            out_d[g * 128:(g + 1) * 128, lo:hi],
                            out_sb[:, lo:hi],
                        )
    nc.compile()
    return nc


def _build_colmap():
    """[NWIN, 4] int64 window->shard-columns map, -1 marks padding."""
    cm = np.full((NWIN, 4), -1, np.int64)
    for r in range(NREG + 1):
        base = r * REGION
        n = _region_wins(r)
        wo = int(WOFF[r])
        route = ROUTES[r] if r < NREG else "A"
        j = np.arange(n)[:, None]
        if route == "A":
            cm[wo:wo + n] = base + 4 * j + np.arange(4)[None, :]
        elif route == "C":
            cm[wo:wo + n, :2] = base + j + np.array([0, REGION // 2])[None, :]
        else:  # D
            cm[wo:wo + n, :1] = base + j
    cm[cm >= VSHARD] = -1  # zero-padded W columns are never candidates
    return cm


_COLMAP = _build_colmap()


def _topk_rows(vals, gidx, k):
    """Per-row top-k ordered like jax.lax.top_k: value desc, index asc."""
    order = np.lexsort((gidx, -vals), axis=-1)[:, :k]
    return (
        np.take_along_axis(gidx, order, axis=1),
        np.take_along_axis(vals, order, axis=1),
    )


def kernel(x: np.ndarray, W: np.ndarray, topk) -> np.ndarray:
    global LAST_RESULTS, _CACHED_NC
    import os

    import ml_dtypes

    from concourse.bass_utils import run_bass_kernel_spmd

    assert x.shape == (B, D) and W.shape == (VOCAB, D)
    assert int(topk) == TOPK
    x = np.ascontiguousarray(np.asarray(x, dtype=np.float32))
    W = np.ascontiguousarray(np.asarray(W, dtype=np.float32))

    if _CACHED_NC is None:
        _CACHED_NC = build_kernel()
    nc = _CACHED_NC

    xt = np.ascontiguousarray(x.T).astype(ml_dtypes.float8_e4m3)
    in_maps = []
    for i in range(NCORES):
        wt_i = np.ascontiguousarray(
            W[i * VSHARD:(i + 1) * VSHARD].T
        ).astype(ml_dtypes.float8_e4m3)
        in_maps.append({"wt": wt_i, "xt": xt})

    LAST_RESULTS = run_bass_kernel_spmd(
        nc,
        in_maps,
        core_ids=list(range(NCORES)),
        trace=bool(int(os.environ.get("KERNEL_TRACE", "0"))),
    )
    results = LAST_RESULTS.results

    # [B, 8*NWIN] device window maxima, f32
    wm = np.concatenate(
        [np.asarray(results[i]["out_win"]).astype(np.float32)
         for i in range(NCORES)], axis=1,
    )
    nwin_all = NCORES * NWIN

    # Per-row window selection: everything >= (128th-largest window max - MARGIN)
    kth = np.partition(wm, nwin_all - TOPK, axis=1)[:, nwin_all - TOPK]
    tau = kth - MARGIN
    counts = (wm >= tau[:, None]).sum(axis=1)
    K = int(min(max(int(counts.max()), TOPK + 64), 6144))
    topw = np.argpartition(-wm, K - 1, axis=1)[:, :K]  # [B, K] window ids

    core_id = topw // NWIN
    wi = topw % NWIN
    cols = _COLMAP[wi]  # [B, K, 4], -1 = pad
    pad = cols < 0
    cand = (np.where(pad, 0, cols) + core_id[..., None] * VSHARD).reshape(B, K * 4)

    # Exact f64 re-rank of the candidate columns (pads scored -inf).
    x64 = x.astype(np.float64)
    W64 = W.astype(np.float64)
    exact = np.empty((B, K * 4), np.float64)
    STEP = 64
    for r0 in range(0, B, STEP):
        r1 = r0 + STEP
        gW = W64[cand[r0:r1]]  # [STEP, K*4, D]
        exact[r0:r1] = np.einsum("bjd,bd->bj", gW, x64[r0:r1])
    exact[pad.reshape(B, K * 4)] = -np.inf

    gidx_top, vals_top = _topk_rows(exact, cand, TOPK)

    # Exactness guards.
    t128 = vals_top[:, -1]
    dev_wmax = np.take_along_axis(wm, topw, axis=1)
    true_wmax = exact.reshape(B, K, 4).max(axis=2)
    err = np.abs(dev_wmax - true_wmax).max(axis=1)
    bad = (
        (err > EPS_BOUND)
        | (tau + EPS_BOUND > t128)
        | (counts > K)
    )
    if os.environ.get("KERNEL_DEBUG"):
        print(f"[kernel] K={K} counts max={counts.max()} "
              f"err max={err.max():.4f} bad rows={int(bad.sum())}")
    for r in np.flatnonzero(bad):
        s = x64[r] @ W64.T
        gidx_top[r] = np.lexsort((np.arange(VOCAB), -s))[:TOPK]

    return gidx_top.astype(np.int32)



# revision 14
# speedup vs baseline: 1.1016x; 1.0087x over previous
"""Distributed exact inner-product top-k (brute-force kNN) on 8 TRN2 NeuronCores.

Sharding: codebook W is split row-wise into 8 shards of 25000 (one per core);
x is replicated.  Host pre-transposes both to bf16: the PE streams 1 output
column/cycle regardless of dtype at contraction 128 (fp8 DoubleRow only
pays off at contraction 256 - measured), so bf16 is free accuracy.

Device kernel (SPMD, identical graph per core, no collectives):
  - per 1024-col region (2 PSUM banks, 4 in flight so both drain engines
    always have a ready region): 2x bf16 matmuls [128 contraction,
    512 cols] into PSUM (f32)
  - each region is drained by one of the only two engines that can read
    PSUM, statically balanced by their clocks and overheads (measured
    1090ns/region DVE vs 1038ns Act):
      D: DVE windowed tensor_reduce(max) w=4 -> bf16 window maxima
         (even regions, 12 per group)
      A: Act copy PSUM -> fp8e4 raw scores, window-1 (odd regions + the
         424-col tail)
  - per-row outputs: 3072 bf16 w4-maxima + 12712 fp8 raw scores, DMA'd in
    multi-region chunks alternating between the SP and gpsimd DMA queues

Host merge (the all-gather + final top-k of the distributed ANN pattern):
  - per row, select every window whose (value + its route's EPS) clears
    (128th-largest window value - EPSMAX - slack); gather member columns
  - exact f64 re-rank of the candidates; final top-128 ordered like
    jax.lax.top_k (value desc, index asc)
  - exactness guard: containment holds if |device value - exact window max|
    <= EPS_route for every window that can matter; all such windows are
    selected, EPS is validated on them per-run, and violating rows
    (expected none) are recomputed exactly.  bf16 inputs keep the gap tiny
    (bf16-out windows ~0.25, fp8e4-out windows ~2.2 at |s|~45), so margins
    and candidate counts stay small and the host merge is cheap.
"""

import numpy as np

B = 1024
D = 128
VOCAB = 200000
NCORES = 8
VSHARD = VOCAB // NCORES  # 25000
REG = 1024  # 2 PSUM banks of f32
NREG = 24  # full regions per shard
TAIL = VSHARD - NREG * REG  # 424 -> Act
TOPK = 128
NGRP = B // 128

# Region schedule per group: Act fp8 copy ("A") on even regions plus 22
# and 23, DVE w4 reduce ("D") on odd regions plus the cheap 424-col tail.
# Act (the busier engine) starts on region 0 so it never waits; the
# measured busies land at DVE ~13.1us vs Act ~13.5us per group.
ROUTES = ["A" if (r % 2 == 0 or r >= 22) else "D" for r in range(NREG)]
ROUTES.append("D")  # tail -> DVE (cheap 505ns reduce)
W4_SEGS = [(r * REG, REG) for r in range(NREG) if ROUTES[r] == "D"] + [
    (NREG * REG, TAIL)
]
W1_SEGS = [(r * REG, REG) for r in range(NREG) if ROUTES[r] == "A"]
NW4 = sum(n // 4 for _, n in W4_SEGS)  # 3072
NW1 = sum(n for _, n in W1_SEGS)  # 12712
W4_OFF = np.concatenate([[0], np.cumsum([n // 4 for _, n in W4_SEGS])])
W1_OFF = np.concatenate([[0], np.cumsum([n for _, n in W1_SEGS])])

# |device window value - exact window max| bounds, validated at runtime:
# fp8e4 W quantization noise (x stays bf16) plus output quantization
# (bf16 ~0.2 for route D, fp8e4 ~2.2 for route A at |s|~45).
EPS4 = 3.4
EPS1 = 5.8
EPSMAX = EPS1
SLACK = 0.3

LAST_RESULTS = None  # BassKernelResults of the most recent run (for profiling)
_CACHED_NC = None


def build_kernel():
    import concourse.bass as bass  # noqa: F401
    import concourse.tile as tile
    from concourse import bacc, mybir

    F32 = mybir.dt.float32
    BF16 = mybir.dt.bfloat16
    FP8 = mybir.dt.float8e4
    AX = mybir.AxisListType.X
    MAX = mybir.AluOpType.max
    COPY = mybir.ActivationFunctionType.Copy

    nc = bacc.Bacc("TRN2", target_bir_lowering=False, debug=False)
    wt_d = nc.dram_tensor("wt", [D, VSHARD], FP8, kind="ExternalInput")
    xt_d = nc.dram_tensor("xt", [D, B], BF16, kind="ExternalInput")
    out4_d = nc.dram_tensor("out_w4", [B, NW4], BF16, kind="ExternalOutput")
    out1_d = nc.dram_tensor("out_w1", [B, NW1], FP8, kind="ExternalOutput")

    with tile.TileContext(nc) as tc:
        with (
            tc.tile_pool(name="wt", bufs=1) as wt_pool,
            tc.tile_pool(name="xt", bufs=1) as xt_pool,
            tc.tile_pool(name="psum", bufs=4, space="PSUM") as psum_pool,
            tc.tile_pool(name="out4", bufs=4) as out4_pool,
            tc.tile_pool(name="out1", bufs=4) as out1_pool,
        ):
            wt_sb = wt_pool.tile([D, VSHARD], FP8)
            xt_sb = xt_pool.tile([D, B], BF16)
            # xt on SP, first W slab concurrently on the gpsimd queue.
            nc.sync.dma_start(xt_sb[:], xt_d[:])
            slabs = [512] * 8 + [1024] * 20 + [424]
            assert sum(slabs) == VSHARD
            lo = 0
            for s, w in enumerate(slabs):
                eng = nc.gpsimd if s % 2 == 0 else nc.sync
                eng.dma_start(wt_sb[:, lo:lo + w], wt_d[:, lo:lo + w])
                lo += w

            # Per-region output window offsets (segments are column-sorted
            # and regions are processed in column order, so offsets align).
            o4_of = {}
            o1_of = {}
            for si, (slo, n) in enumerate(W4_SEGS):
                o4_of[slo] = int(W4_OFF[si])
            for si, (slo, n) in enumerate(W1_SEGS):
                o1_of[slo] = int(W1_OFF[si])

            # DMA-out cuts: (after_region, stream, win_lo, win_hi)
            def done4(r):
                return sum(
                    n // 4 for slo, n in W4_SEGS if slo < (r + 1) * REG
                )

            def done1(r):
                return sum(n for slo, n in W1_SEGS if slo < (r + 1) * REG)

            cuts = []
            prev1 = 0
            for cr in (2, 6, 10, 14, 18, 23):
                hi = done1(cr)
                cuts.append((cr, 1, prev1, hi))
                prev1 = hi
            prev4 = 0
            for cr in (13, NREG):
                hi = done4(cr)
                cuts.append((cr, 4, prev4, hi))
                prev4 = hi

            for g in range(NGRP):
                out4_sb = out4_pool.tile([128, NW4], BF16, tag="out4")
                out1_sb = out1_pool.tile([128, NW1], FP8, tag="out1")
                xg = xt_sb[:, g * 128:(g + 1) * 128]
                for r in range(NREG + 1):
                    base = r * REG
                    w_cols = REG if r < NREG else TAIL
                    route = ROUTES[r]
                    ps = psum_pool.tile([128, REG], F32)
                    for k in range(0, w_cols, 512):
                        kw = min(512, w_cols - k)
                        nc.tensor.matmul(
                            ps[:, k:k + kw],
                            xg,
                            wt_sb[:, base + k:base + k + kw],
                            start=True, stop=True,
                        )
                    if route == "D":
                        o4 = o4_of[base]
                        nc.vector.tensor_reduce(
                            out4_sb[:, o4:o4 + w_cols // 4],
                            ps[:, :w_cols].rearrange("p (n w) -> p n w", w=4),
                            axis=AX, op=MAX,
                        )
                    else:
                        o1 = o1_of[base]
                        nc.scalar.activation(
                            out1_sb[:, o1:o1 + w_cols],
                            ps[:, :w_cols],
                            COPY,
                        )
                    for ci, (cr, which, wlo, whi) in enumerate(cuts):
                        if cr != r:
                            continue
                        eng = nc.sync if (g + ci) % 2 == 0 else nc.gpsimd
                        src_t = out4_sb if which == 4 else out1_sb
                        dst = out4_d if which == 4 else out1_d
                        eng.dma_start(
                            dst[g * 128:(g + 1) * 128, wlo:whi],
                            src_t[:, wlo:whi],
                        )
    nc.compile()
    return nc


def _build_maps():
    """Per-window candidate columns and EPS.

    Returns (colmap [NWIN, 4] int64 with -1 pads, eps [NWIN] f32) where
    window order is [all w4 windows, all w1 windows] per core.
    """
    nwin = NW4 + NW1
    cm = np.full((nwin, 4), -1, np.int64)
    eps = np.empty(nwin, np.float32)
    for si, (lo, n) in enumerate(W4_SEGS):
        o = int(W4_OFF[si])
        j = np.arange(n // 4)[:, None]
        cm[o:o + n // 4] = lo + 4 * j + np.arange(4)[None, :]
    eps[:NW4] = EPS4
    for si, (lo, n) in enumerate(W1_SEGS):
        o = NW4 + int(W1_OFF[si])
        cm[o:o + n, 0] = lo + np.arange(n)
    eps[NW4:] = EPS1
    return cm, eps


_COLMAP, _WEPS = _build_maps()


def _topk_rows(vals, gidx, k):
    """Per-row top-k ordered like jax.lax.top_k: value desc, index asc."""
    order = np.lexsort((gidx, -vals), axis=-1)[:, :k]
    return (
        np.take_along_axis(gidx, order, axis=1),
        np.take_along_axis(vals, order, axis=1),
    )


def kernel(x: np.ndarray, W: np.ndarray, topk) -> np.ndarray:
    global LAST_RESULTS, _CACHED_NC
    import os

    import ml_dtypes

    from concourse.bass_utils import run_bass_kernel_spmd

    assert x.shape == (B, D) and W.shape == (VOCAB, D)
    assert int(topk) == TOPK
    x = np.ascontiguousarray(np.asarray(x, dtype=np.float32))
    W = np.ascontiguousarray(np.asarray(W, dtype=np.float32))

    if _CACHED_NC is None:
        _CACHED_NC = build_kernel()
    nc = _CACHED_NC

    xt = np.ascontiguousarray(x.T).astype(ml_dtypes.bfloat16)
    in_maps = []
    for i in range(NCORES):
        wt_i = np.ascontiguousarray(
            W[i * VSHARD:(i + 1) * VSHARD].T
        ).astype(ml_dtypes.float8_e4m3)
        in_maps.append({"wt": wt_i, "xt": xt})

    LAST_RESULTS = run_bass_kernel_spmd(
        nc,
        in_maps,
        core_ids=list(range(NCORES)),
        trace=bool(int(os.environ.get("KERNEL_TRACE", "0"))),
    )
    results = LAST_RESULTS.results

    # [B, 8*(NW4+NW1)] device window values, f32
    nwin = NW4 + NW1
    wm = np.empty((B, NCORES * nwin), np.float32)
    for i in range(NCORES):
        wm[:, i * nwin:i * nwin + NW4] = np.asarray(
            results[i]["out_w4"]).astype(np.float32)
        wm[:, i * nwin + NW4:(i + 1) * nwin] = np.asarray(
            results[i]["out_w1"]).astype(np.float32)
    nwin_all = NCORES * nwin
    weps_all = np.tile(_WEPS, NCORES)

    # Per-row selection on adjusted values v' = v + eps_w:
    # keep windows with v' >= kth_dev - EPSMAX - SLACK.
    wma = wm + weps_all[None, :]
    kth = np.partition(wm, nwin_all - TOPK, axis=1)[:, nwin_all - TOPK]
    tau = kth - EPSMAX - SLACK
    counts = (wma >= tau[:, None]).sum(axis=1)
    K = int(min(max(int(counts.max()), TOPK + 64), 4096))
    topw = np.argpartition(-wma, K - 1, axis=1)[:, :K]  # [B, K] window ids

    core_id = topw // nwin
    wi = topw % nwin
    cols = _COLMAP[wi]  # [B, K, 4], -1 = pad
    pad = cols < 0
    cand = (np.where(pad, 0, cols) + core_id[..., None] * VSHARD).reshape(B, K * 4)

    # Exact f64 re-rank of the candidate columns (pads scored -inf).
    x64 = x.astype(np.float64)
    W64 = W.astype(np.float64)
    exact = np.empty((B, K * 4), np.float64)
    STEP = 64
    for r0 in range(0, B, STEP):
        r1 = r0 + STEP
        gW = W64[cand[r0:r1]]  # [STEP, K*4, D]
        exact[r0:r1] = np.einsum("bjd,bd->bj", gW, x64[r0:r1])
    exact[pad.reshape(B, K * 4)] = -np.inf

    # Rank on f32-rounded scores so near-ties break the same way as the
    # f32 reference (top_k on an f32 score matrix, ties by index asc).
    gidx_top, vals_top = _topk_rows(
        exact.astype(np.float32).astype(np.float64), cand, TOPK)

    # Exactness guards: EPS must hold on every selected window (any window
    # that can contain a true top-128 column is selected), and the
    # selection count must fit in K.
    dev_w = np.take_along_axis(wm, topw, axis=1)
    true_w = exact.reshape(B, K, 4).max(axis=2)
    werr = np.abs(dev_w - true_w)
    sel_eps = weps_all[topw]
    err_excess = (werr - sel_eps).max(axis=1)
    bad = (err_excess > 0) | (counts > K)
    if os.environ.get("KERNEL_DEBUG"):
        w4mask = (topw % nwin) < NW4
        e4 = werr[w4mask].max() if w4mask.any() else 0.0
        e1 = werr[~w4mask].max() if (~w4mask).any() else 0.0
        print(f"[kernel] K={K} counts max={counts.max()} "
              f"err4 max={e4:.3f} err1 max={e1:.3f} bad rows={int(bad.sum())}")
    for r in np.flatnonzero(bad):
        s = x64[r] @ W64.T
        gidx_top[r] = np.lexsort((np.arange(VOCAB), -s))[:TOPK]

    return gidx_top.astype(np.int32)
